# revision 36
# baseline (speedup 1.0000x reference)
"""Multi-head causal attention (B=2,T=2048,C=1024,H=16,Dh=64) on 8 trn2 cores.

Sharding: tensor-parallel over heads - core c owns heads (2c, 2c+1).
Per core: QKV projections for its 128 q/k/v columns, causal flash attention
for its 2 heads x 2 batches, then an AllToAll reshard (heads-sharded ->
token-sharded) and the output projection for its own tokens.

v2 schedule (vs the 196us baseline):
- strips are 256 tokens; software-pipelined score->exp->AV with depth-1 skew
  so PE and Act overlap per tile.
- a "filler" queue feeds PE bubbles during Act-bound attention: QKV of the
  other batch and projection units run inside attention windows.
- each batch's reshard is split into two half collectives (token ownership
  interleaved at 128-token granularity); after the last attention strip only
  one small collective plus a quarter of the projection remains, and the
  second-to-last projection quarter runs inside that collective's shadow.
- collective APs keep their natural [8,128,128] chunk form.
- v transposes ride the DMA xbar instead of PE.
- QKV runs by token quarters so attention starts a few us into the kernel.
"""
import collections

import numpy as np
import ml_dtypes

import concourse.bass as bass
import concourse.mybir as mybir
import concourse.tile as tile
from concourse.bass_utils import run_bass_kernel_spmd
from concourse.masks import make_identity
from concourse.vector_clock import ScopedClock

BF16 = mybir.dt.bfloat16
F32 = mybir.dt.float32

B, T, C = 2, 2048, 1024
H, DH = 16, 64
NCORES = 8
HPC = 128      # head-columns per core (2 heads x 64)
NI = 256       # strip width (query tokens)
NJ = 128       # key-tile width
NSTRIP = T // NI          # 8 strips per batch
NJT = T // NJ             # 16 j-tiles per batch
NCC = C // 128            # 8 contraction chunks
NQ = 4                    # token quarters for QKV
QW = T // NQ              # 512 tokens per quarter
SCALE = DH ** -0.5


class TileContextP(tile.TileContext):
    """This walrus build caps sync waits at 1 per instruction (2 for
    EventSemaphore). Tile can emit more. Legalize by spilling excess waits
    onto same-engine nops emitted just before the instruction, and do the
    same for the kernel-tail drain."""

    def _commit_instruction(self, inst, lazy_reg_writes: bool = True):
        si = getattr(inst, "sync_info", None)
        if si is not None and si.on_wait:
            cap = 2 if isinstance(inst, mybir.InstEventSemaphore) else 1
            if len(si.on_wait) > cap:
                waits = list(si.on_wait)
                keep, spill = waits[:cap - 1] if cap > 1 else [], waits[cap - 1:]
                # keep the last wait on the inst, spill the rest
                spill, last = spill[:-1], spill[-1:]
                for w in spill:
                    nop = mybir.InstNoOp(
                        name=self.nc.get_next_instruction_name(),
                        engine=inst.engine,
                        sync_info=mybir.SyncInfo(on_wait=[w], on_update=[]),
                        bass_nofuse=True,
                    )
                    self._add_instruction(nop)
                si.on_wait = keep + last
        return super()._commit_instruction(inst, lazy_reg_writes)

    def _drain_and_barrier(self, tick_clock, wait_clock):
        probe = self.nc.sync.nop()
        wait_clock.add_sem_waits(
            probe.ins, ScopedClock({None: tick_clock.global_clock})
        )
        waits = list(probe.ins.sync_info.on_wait) if probe.ins.sync_info else []
        if probe.ins.sync_info:
            probe.ins.sync_info.on_wait = []
        for w in waits:
            n = self.nc.sync.nop()
            si = n.ins.sync_info
            if si is None:
                n.ins.sync_info = mybir.SyncInfo(on_wait=[w], on_update=[])
            else:
                si.on_wait = [w]
        self.nc.sync.drain()
        self.nc.all_engine_barrier()
        assert self.sems is not None
        popped = self.nc._tile_sem_poison_stack.pop()
        assert popped is self._sem_poison
        self.nc.clear_and_free_semaphores(list(self.sems.allocated().values()))
        self.nc.all_engine_barrier()


def build_nc():
    nc = bass.Bass()
    xT_h = nc.dram_tensor("xT", [B, C, T], BF16, kind="ExternalInput")
    wq_h = nc.dram_tensor("wq", [C, HPC], BF16, kind="ExternalInput")
    wk_h = nc.dram_tensor("wk", [C, HPC], BF16, kind="ExternalInput")
    wv_h = nc.dram_tensor("wv", [C, HPC], BF16, kind="ExternalInput")
    wp_h = nc.dram_tensor("wp", [C, C], BF16, kind="ExternalInput")
    mk_h = nc.dram_tensor("masks", [2, NJ, NI], BF16, kind="ExternalInput")
    # y_out[:, b, hh, :] = out rows x tokens [1024*hh + 128*core, +128) of b
    y_h = nc.dram_tensor("y_out", [C, B * 2 * NJ], F32, kind="ExternalOutput")
    # reshard buffers, one per (batch, half): slot j of a2a_in = this core's
    # head-pair rows for tokens [1024*hh + 128*j, +128)
    rsc_h = nc.dram_tensor("rscratch", [B * NSTRIP, 2 * NI], F32)
    a2a_in = [[nc.dram_tensor(f"a2a_in{b}_{hh}", [NCORES, HPC, NJ], BF16)
               for hh in range(2)] for b in range(B)]
    a2a_out = [[nc.dram_tensor(f"a2a_out{b}_{hh}", [NCORES, HPC, NJ], BF16)
                for hh in range(2)] for b in range(B)]

    with TileContextP(nc) as tc, \
         tc.tile_pool(name="singles", bufs=1) as singles, \
         tc.tile_pool(name="xqp", bufs=6) as xqp, \
         tc.tile_pool(name="qkp", bufs=1) as qkp, \
         tc.tile_pool(name="vtp", bufs=2) as vtp, \
         tc.tile_pool(name="weip", bufs=5) as weip, \
         tc.tile_pool(name="attp", bufs=3) as attp, \
         tc.tile_pool(name="smallp", bufs=2) as smallp, \
         tc.tile_pool(name="rhsp", bufs=2) as rhsp, \
         tc.tile_pool(name="yop", bufs=4) as yop, \
         tc.tile_pool(name="scops", bufs=2, space="PSUM") as scops, \
         tc.tile_pool(name="oaps", bufs=2, space="PSUM") as oaps, \
         tc.tile_pool(name="filps", bufs=2, space="PSUM") as filps:

        # ---- weights first (gate the first matmuls), then x(b0), masks
        wq = singles.tile([128, NCC, HPC], BF16, name="wq_s", tag="wq_s")
        wk = singles.tile([128, NCC, HPC], BF16, name="wk_s", tag="wk_s")
        wv = singles.tile([128, NCC, HPC], BF16, name="wv_s", tag="wv_s")
        nc.scalar.dma_start(out=wq, in_=wq_h.rearrange("(n p) m -> p n m", p=128))

        xsrc = [xT_h[b].rearrange("(n p) t -> p n t", p=128) for b in range(B)]
        xt = {}

        def load_x_quarter(b, tq):
            xq = xqp.tile([128, NCC, QW], BF16, name="xq", tag="xq")
            for cc in range(NCC):
                nc.sync.dma_start(
                    out=xq[:, cc, :],
                    in_=xsrc[b][:, cc, tq * QW:(tq + 1) * QW],
                )
            xt[(b, tq)] = xq

        load_x_quarter(0, 0)
        nc.scalar.dma_start(out=wk, in_=wk_h.rearrange("(n p) m -> p n m", p=128))
        nc.scalar.dma_start(out=wv, in_=wv_h.rearrange("(n p) m -> p n m", p=128))
        masks = singles.tile([128, 2, NI], BF16, name="masks_s", tag="masks_s")
        nc.scalar.dma_start(out=masks, in_=mk_h.rearrange("d p i -> p d i"))
        for tq in range(1, NQ):
            load_x_quarter(0, tq)
        wp = singles.tile([128, NCC, C], BF16, name="wp_s", tag="wp_s")
        nc.sync.dma_start(out=wp, in_=wp_h.rearrange("(n p) m -> p n m", p=128))

        ident = singles.tile([128, 128], BF16, name="ident_s", tag="ident_s")
        make_identity(nc, ident)
        ones64 = singles.tile([1, 64], BF16, name="ones64", tag="ones64")
        nc.vector.memset(ones64, 1.0)

        qt = {b: qkp.tile([64, 2, T], BF16, name=f"qt{b}", tag=f"qt{b}")
              for b in range(B)}
        kt = {b: qkp.tile([64, 2, T], BF16, name=f"kt{b}", tag=f"kt{b}")
              for b in range(B)}
        vaug = {b: qkp.tile([128, NJT, 130], BF16, name=f"vaug{b}", tag=f"vaug{b}")
                for b in range(B)}
        for b in range(B):
            nc.vector.memset(vaug[b], 1.0)

        def emit_qkv_unit(b, tq, which):
            """One (w, quarter) projection: 8 accum matmuls + copy-out.
            For v, also xbar-transpose into vaug."""
            xq = xt[(b, tq)]
            w_t = {"q": wq, "k": wk, "v": wv}[which]
            ps = filps.tile([128, QW], F32, name="psq", tag="q")
            for cc in range(NCC):
                nc.tensor.matmul(ps, w_t[:, cc, :], xq[:, cc, :],
                                 start=(cc == 0), stop=(cc == NCC - 1))
            if which == "q":
                for h in range(2):
                    nc.vector.tensor_copy(
                        qt[b][:, h, tq * QW:(tq + 1) * QW],
                        ps[h * 64:(h + 1) * 64, :])
            elif which == "k":
                for h in range(2):
                    nc.scalar.copy(
                        kt[b][:, h, tq * QW:(tq + 1) * QW],
                        ps[h * 64:(h + 1) * 64, :])
            else:
                vt = vtp.tile([128, QW], BF16, name="vt", tag="vt")
                nc.vector.tensor_copy(vt, ps)
                for j in range(QW // NJ):
                    jt = tq * (QW // NJ) + j
                    ptr = filps.tile([128, NJ], BF16, name="ptr", tag="q")
                    nc.tensor.transpose(ptr, vt[:, j * NJ:(j + 1) * NJ], ident)
                    nc.vector.tensor_copy(vaug[b][:, jt, 0:64], ptr[:, 0:64])
                    nc.vector.tensor_copy(vaug[b][:, jt, 65:129], ptr[:, 64:128])

        # ---- filler machinery: labeled closures emitted into PE bubbles
        fillers = collections.deque()
        tile_ctr = {"n": 0}

        def pop_filler():
            tile_ctr["n"] += 1

        def flush_until(label):
            pass

        def flush_fillers():
            while fillers:
                _, fn = fillers.popleft()
                fn()

        # ---- projection for (b, half): 8 psum tiles contracted over 8 peers.
        # Emitted inline at points where the feeding collective is already
        # done (a proj matmul waiting on its rhs DMA would head-of-line
        # block the whole PE queue).
        def emit_proj(b, hh):
            rhs_tiles = []
            for j in range(NCORES):
                rt_ = rhsp.tile([128, NJ], BF16, name="rt", tag="rt", bufs=16)
                nc.sync.dma_start(out=rt_, in_=a2a_out[b][hh][j])
                rhs_tiles.append(rt_)
            for nt in range(NCC):
                py = filps.tile([128, NJ], F32, name="py", tag="q")
                for j in range(NCORES):
                    nc.tensor.matmul(py, wp[:, j, nt * 128:(nt + 1) * 128],
                                     rhs_tiles[j],
                                     start=(j == 0), stop=(j == NCORES - 1))
                yo = yop.tile([128, NJ], F32, name="yo", tag="yo")
                nc.vector.tensor_copy(yo, py)
                nc.sync.dma_start(
                    out=y_h[nt * 128:(nt + 1) * 128,
                            (b * 2 + hh) * NJ:(b * 2 + hh + 1) * NJ], in_=yo)

        # ---- one attention strip, software-pipelined with filler slots
        def emit_strip(b, st):
            i0 = st * NI
            njt = 2 * (st + 1)
            oaug = oaps.tile([65, 1024], F32, name="oaug", tag="oaug")
            weis = {}

            def emit_sco_exp(jt):
                j0 = jt * NJ
                d = jt - (njt - 2)
                lo = 128 if d == 1 else 0
                sco = scops.tile([128, 2 * NI], F32, name="sco", tag="sco")
                for h in range(2):
                    nc.tensor.matmul(
                        sco[:, h * NI + lo:(h + 1) * NI],
                        kt[b][:, h, j0:j0 + NJ],
                        qt[b][:, h, i0 + lo:i0 + NI],
                        start=True, stop=True,
                    )
                wei = weip.tile([128, 2 * NI], BF16, name="wei", tag="wei")
                if d < 1:
                    nc.scalar.activation(wei, sco,
                                         mybir.ActivationFunctionType.Exp,
                                         scale=SCALE)
                else:
                    for h in range(2):
                        nc.scalar.activation(
                            wei[:, h * NI + lo:(h + 1) * NI],
                            sco[:, h * NI + lo:(h + 1) * NI],
                            mybir.ActivationFunctionType.Exp, scale=SCALE)
                if d >= 0:
                    for h in range(2):
                        nc.vector.tensor_mul(
                            wei[:, h * NI + lo:(h + 1) * NI],
                            wei[:, h * NI + lo:(h + 1) * NI],
                            masks[:, d, lo:],
                        )
                weis[jt] = (wei, lo)

            def emit_av(jt):
                wei, lo = weis.pop(jt)
                for h in range(2):
                    nc.tensor.matmul(
                        oaug[:, h * 512 + lo:h * 512 + NI],
                        vaug[b][:, jt, h * 65:(h + 1) * 65],
                        wei[:, h * NI + lo:(h + 1) * NI],
                        start=(jt == 0), stop=(jt == njt - 1),
                    )

            for jt in range(njt):
                emit_sco_exp(jt)
                if jt >= 2:
                    pop_filler()
                    emit_av(jt - 2)
            emit_av(njt - 2)
            emit_av(njt - 1)

            # epilogue: normalize via reciprocal + DRAM-bounce broadcast
            sl = b * NSTRIP + st
            r = smallp.tile([1, 2 * NI], F32, name="r", tag="r")
            for h in range(2):
                nc.vector.reciprocal(r[:, h * NI:(h + 1) * NI],
                                     oaug[64:65, h * 512:h * 512 + NI])
            nc.sync.dma_start(out=rsc_h[sl:sl + 1, :], in_=r)
            rbs = smallp.tile([64, 2 * NI], F32, name="rbs", tag="rbs")
            bcast = bass.AP(
                tensor=rsc_h.tensor if hasattr(rsc_h, "tensor") else rsc_h,
                offset=sl * 2 * NI,
                ap=[[0, 64], [1, 2 * NI]],
            )
            nc.sync.dma_start(out=rbs, in_=bcast)
            att = attp.tile([128, NI], BF16, name="att", tag="att")
            for h in range(2):
                nc.vector.tensor_mul(
                    att[h * 64:(h + 1) * 64, :],
                    oaug[0:64, h * 512:h * 512 + NI],
                    rbs[:, h * NI:(h + 1) * NI],
                )
            # ship the two 128-token chunks to their reshard slots
            hh = 0 if st < 4 else 1
            for hf in range(2):
                nc.sync.dma_start(
                    out=a2a_in[b][hh][(2 * st + hf) % 8],
                    in_=att[:, hf * NJ:(hf + 1) * NJ],
                )

        def emit_a2a(b, hh):
            nc.gpsimd.collective_compute(
                "AllToAll",
                mybir.AluOpType.bypass,
                replica_groups=[list(range(NCORES))],
                ins=[a2a_in[b][hh][:, :, :].opt()],
                outs=[a2a_out[b][hh][:, :, :].opt()],
            )

        # ================= main schedule =================
        # All QKV beyond (0,q0) rides the filler queue in dependency order:
        # (0,q1..q3) then (1,q0),(1,q1); b1's late quarters pop inside b1's
        # own early strips. flush_until() guarantees a quarter is emitted
        # before the first strip that reads it.
        def enq_qkv(b, tq):
            for which in ("q", "k", "v"):
                emit_qkv_unit(b, tq, which)

        for which in ("q", "k", "v"):
            emit_qkv_unit(0, 0, which)
        for st in range(NSTRIP):
            if st < 3:
                enq_qkv(0, st + 1)
            if 2 <= st <= 3:
                load_x_quarter(1, st - 2)
            if 3 <= st <= 4:
                enq_qkv(1, st - 3)
            if st >= 2:
                flush_until((0, st // 2))
            emit_strip(0, st)
            if st == 3:
                emit_a2a(0, 0)
        emit_a2a(0, 1)    # issue before the QKV(b1) flush: b0 data is ready
        flush_until((1, 0))
        flush_until((1, 1))

        for st in range(NSTRIP):
            if st == 0:
                load_x_quarter(1, 2)
                enq_qkv(1, 2)
            if st == 1:
                load_x_quarter(1, 3)
                enq_qkv(1, 3)
            if st >= 4:
                flush_until((1, st // 2))
            emit_strip(1, st)
            if st == 2:
                emit_proj(0, 0)   # cc(0,0) completed during b0's tail
            if st == 3:
                emit_a2a(1, 0)
            if st == 5:
                emit_proj(0, 1)   # cc(0,1) completed during b1's early strips
        # tail: proj(1,0) overlaps the last collective, then proj(1,1)
        emit_proj(1, 0)
        emit_a2a(1, 1)
        emit_proj(1, 1)
    return nc


_NC_CACHE = {}


def _get_nc():
    if "nc" not in _NC_CACHE:
        _NC_CACHE["nc"] = build_nc()
    return _NC_CACHE["nc"]


def _host_masks():
    jl = np.arange(NJ)[:, None]
    il = np.arange(NI)[None, :]
    return np.stack([(il >= jl + d * 128) for d in range(2)]).astype(ml_dtypes.bfloat16)


def kernel(x, Wk, Wq, Wv, Wp, bp):
    x = np.asarray(x)
    xT = np.ascontiguousarray(x.transpose(0, 2, 1)).astype(ml_dtypes.bfloat16)
    wpb = np.asarray(Wp).astype(ml_dtypes.bfloat16)
    masks = _host_masks()
    in_maps = []
    for c in range(NCORES):
        cs = slice(c * HPC, (c + 1) * HPC)
        in_maps.append({
            "xT": xT,
            "wq": np.ascontiguousarray(Wq[:, cs]).astype(ml_dtypes.bfloat16),
            "wk": np.ascontiguousarray(Wk[:, cs]).astype(ml_dtypes.bfloat16),
            "wv": np.ascontiguousarray(Wv[:, cs]).astype(ml_dtypes.bfloat16),
            "wp": wpb,
            "masks": masks,
        })
    res = run_bass_kernel_spmd(_get_nc(), in_maps, list(range(NCORES)))
    # core c's y_out[:, b, hh, :] covers batch-b tokens [1024*hh+128c, +128)
    yT = np.zeros((B, C, T), np.float32)
    for c in range(NCORES):
        yo = res.results[c]["y_out"].reshape(C, B, 2, NJ)
        for b in range(B):
            for hh in range(2):
                t0 = 1024 * hh + 128 * c
                yT[b, :, t0:t0 + 128] = yo[:, b, hh, :]
    y = yT.transpose(0, 2, 1) + np.asarray(bp)[None, None, :]
    return np.ascontiguousarray(y, dtype=np.float32)


# revision 37
# speedup vs baseline: 1.0559x; 1.0559x over previous
"""Multi-head causal attention (B=2,T=2048,C=1024,H=16,Dh=64) on 8 trn2 cores.

Sharding: tensor-parallel over heads - core c owns heads (2c, 2c+1).
Per core: QKV projections for its 128 q/k/v columns, causal flash attention
for its 2 heads x 2 batches, then an AllToAll reshard (heads-sharded ->
token-sharded) and the output projection for its own tokens.

v2 schedule (vs the 196us baseline):
- strips are 256 tokens; software-pipelined score->exp->AV with depth-1 skew
  so PE and Act overlap per tile.
- a "filler" queue feeds PE bubbles during Act-bound attention: QKV of the
  other batch and projection units run inside attention windows.
- each batch's reshard is split into two half collectives (token ownership
  interleaved at 128-token granularity); after the last attention strip only
  one small collective plus a quarter of the projection remains, and the
  second-to-last projection quarter runs inside that collective's shadow.
- collective APs keep their natural [8,128,128] chunk form.
- v transposes ride the DMA xbar instead of PE.
- QKV runs by token quarters so attention starts a few us into the kernel.
"""
import collections

import numpy as np
import ml_dtypes

import concourse.bass as bass
import concourse.mybir as mybir
import concourse.tile as tile
from concourse.bass_utils import run_bass_kernel_spmd
from concourse.masks import make_identity
from concourse.vector_clock import ScopedClock

BF16 = mybir.dt.bfloat16
F32 = mybir.dt.float32

B, T, C = 2, 2048, 1024
H, DH = 16, 64
NCORES = 8
HPC = 128      # head-columns per core (2 heads x 64)
NI = 256       # strip width (query tokens)
NJ = 128       # key-tile width
NSTRIP = T // NI          # 8 strips per batch
NJT = T // NJ             # 16 j-tiles per batch
NCC = C // 128            # 8 contraction chunks
NQ = 4                    # token quarters for QKV
QW = T // NQ              # 512 tokens per quarter
SCALE = DH ** -0.5


class TileContextP(tile.TileContext):
    """This walrus build caps sync waits at 1 per instruction (2 for
    EventSemaphore). Tile can emit more. Legalize by spilling excess waits
    onto same-engine nops emitted just before the instruction, and do the
    same for the kernel-tail drain."""

    def _commit_instruction(self, inst, lazy_reg_writes: bool = True):
        si = getattr(inst, "sync_info", None)
        if si is not None and si.on_wait:
            cap = 2 if isinstance(inst, mybir.InstEventSemaphore) else 1
            if len(si.on_wait) > cap:
                waits = list(si.on_wait)
                keep, spill = waits[:cap - 1] if cap > 1 else [], waits[cap - 1:]
                # keep the last wait on the inst, spill the rest
                spill, last = spill[:-1], spill[-1:]
                for w in spill:
                    nop = mybir.InstNoOp(
                        name=self.nc.get_next_instruction_name(),
                        engine=inst.engine,
                        sync_info=mybir.SyncInfo(on_wait=[w], on_update=[]),
                        bass_nofuse=True,
                    )
                    self._add_instruction(nop)
                si.on_wait = keep + last
        return super()._commit_instruction(inst, lazy_reg_writes)

    def _drain_and_barrier(self, tick_clock, wait_clock):
        probe = self.nc.sync.nop()
        wait_clock.add_sem_waits(
            probe.ins, ScopedClock({None: tick_clock.global_clock})
        )
        waits = list(probe.ins.sync_info.on_wait) if probe.ins.sync_info else []
        if probe.ins.sync_info:
            probe.ins.sync_info.on_wait = []
        for w in waits:
            n = self.nc.sync.nop()
            si = n.ins.sync_info
            if si is None:
                n.ins.sync_info = mybir.SyncInfo(on_wait=[w], on_update=[])
            else:
                si.on_wait = [w]
        self.nc.sync.drain()
        self.nc.all_engine_barrier()
        assert self.sems is not None
        popped = self.nc._tile_sem_poison_stack.pop()
        assert popped is self._sem_poison
        self.nc.clear_and_free_semaphores(list(self.sems.allocated().values()))
        self.nc.all_engine_barrier()


def build_nc():
    nc = bass.Bass()
    xT_h = nc.dram_tensor("xT", [B, C, T], BF16, kind="ExternalInput")
    wq_h = nc.dram_tensor("wq", [C, HPC], BF16, kind="ExternalInput")
    wk_h = nc.dram_tensor("wk", [C, HPC], BF16, kind="ExternalInput")
    wv_h = nc.dram_tensor("wv", [C, HPC], BF16, kind="ExternalInput")
    wp_h = nc.dram_tensor("wp", [C, C], BF16, kind="ExternalInput")
    mk_h = nc.dram_tensor("masks", [2, NJ, NI], BF16, kind="ExternalInput")
    # y_out[:, b, hh, :] = out rows x tokens [1024*hh + 128*core, +128) of b
    y_h = nc.dram_tensor("y_out", [C, B * 2 * NJ], F32, kind="ExternalOutput")
    # reshard buffers, one per (batch, half): slot j of a2a_in = this core's
    # head-pair rows for tokens [1024*hh + 128*j, +128)
    a2a_in = [[nc.dram_tensor(f"a2a_in{b}_{hh}", [NCORES, HPC, NJ], BF16)
               for hh in range(2)] for b in range(B)]
    a2a_out = [[nc.dram_tensor(f"a2a_out{b}_{hh}", [NCORES, HPC, NJ], BF16)
                for hh in range(2)] for b in range(B)]

    with TileContextP(nc) as tc, \
         tc.tile_pool(name="singles", bufs=1) as singles, \
         tc.tile_pool(name="xqp", bufs=6) as xqp, \
         tc.tile_pool(name="qkp", bufs=1) as qkp, \
         tc.tile_pool(name="vtp", bufs=2) as vtp, \
         tc.tile_pool(name="weip", bufs=5) as weip, \
         tc.tile_pool(name="attp", bufs=3) as attp, \
         tc.tile_pool(name="smallp", bufs=2) as smallp, \
         tc.tile_pool(name="rhsp", bufs=2) as rhsp, \
         tc.tile_pool(name="yop", bufs=4) as yop, \
         tc.tile_pool(name="scops", bufs=2, space="PSUM") as scops, \
         tc.tile_pool(name="oaps", bufs=2, space="PSUM") as oaps, \
         tc.tile_pool(name="filps", bufs=2, space="PSUM") as filps:

        # ---- weights first (gate the first matmuls), then x(b0), masks
        wq = singles.tile([128, NCC, HPC], BF16, name="wq_s", tag="wq_s")
        wk = singles.tile([128, NCC, HPC], BF16, name="wk_s", tag="wk_s")
        wv = singles.tile([128, NCC, HPC], BF16, name="wv_s", tag="wv_s")
        nc.scalar.dma_start(out=wq, in_=wq_h.rearrange("(n p) m -> p n m", p=128))

        xsrc = [xT_h[b].rearrange("(n p) t -> p n t", p=128) for b in range(B)]
        xt = {}

        def load_x_quarter(b, tq):
            xq = xqp.tile([128, NCC, QW], BF16, name="xq", tag="xq")
            for cc in range(NCC):
                nc.sync.dma_start(
                    out=xq[:, cc, :],
                    in_=xsrc[b][:, cc, tq * QW:(tq + 1) * QW],
                )
            xt[(b, tq)] = xq

        load_x_quarter(0, 0)
        nc.scalar.dma_start(out=wk, in_=wk_h.rearrange("(n p) m -> p n m", p=128))
        nc.scalar.dma_start(out=wv, in_=wv_h.rearrange("(n p) m -> p n m", p=128))
        masks = singles.tile([128, 2, NI], BF16, name="masks_s", tag="masks_s")
        nc.scalar.dma_start(out=masks, in_=mk_h.rearrange("d p i -> p d i"))
        for tq in range(1, NQ):
            load_x_quarter(0, tq)
        wp = singles.tile([128, NCC, C], BF16, name="wp_s", tag="wp_s")
        nc.sync.dma_start(out=wp, in_=wp_h.rearrange("(n p) m -> p n m", p=128))

        ident = singles.tile([128, 128], BF16, name="ident_s", tag="ident_s")
        make_identity(nc, ident)
        ones64 = singles.tile([1, 64], BF16, name="ones64", tag="ones64")
        nc.vector.memset(ones64, 1.0)

        qt = {b: qkp.tile([64, 2, T], BF16, name=f"qt{b}", tag=f"qt{b}")
              for b in range(B)}
        kt = {b: qkp.tile([64, 2, T], BF16, name=f"kt{b}", tag=f"kt{b}")
              for b in range(B)}
        vaug = {b: qkp.tile([128, NJT, 130], BF16, name=f"vaug{b}", tag=f"vaug{b}")
                for b in range(B)}
        for b in range(B):
            nc.vector.memset(vaug[b], 1.0)

        def emit_qkv_unit(b, tq, which):
            """One (w, quarter) projection: 8 accum matmuls + copy-out.
            For v, also xbar-transpose into vaug."""
            xq = xt[(b, tq)]
            w_t = {"q": wq, "k": wk, "v": wv}[which]
            ps = filps.tile([128, QW], F32, name="psq", tag="q")
            for cc in range(NCC):
                nc.tensor.matmul(ps, w_t[:, cc, :], xq[:, cc, :],
                                 start=(cc == 0), stop=(cc == NCC - 1))
            if which == "q":
                for h in range(2):
                    nc.vector.tensor_copy(
                        qt[b][:, h, tq * QW:(tq + 1) * QW],
                        ps[h * 64:(h + 1) * 64, :])
            elif which == "k":
                for h in range(2):
                    nc.scalar.copy(
                        kt[b][:, h, tq * QW:(tq + 1) * QW],
                        ps[h * 64:(h + 1) * 64, :])
            else:
                vt = vtp.tile([128, QW], BF16, name="vt", tag="vt")
                nc.vector.tensor_copy(vt, ps)
                for j in range(QW // NJ):
                    jt = tq * (QW // NJ) + j
                    ptr = filps.tile([128, NJ], BF16, name="ptr", tag="q")
                    nc.tensor.transpose(ptr, vt[:, j * NJ:(j + 1) * NJ], ident)
                    nc.vector.tensor_copy(vaug[b][:, jt, 0:64], ptr[:, 0:64])
                    nc.vector.tensor_copy(vaug[b][:, jt, 65:129], ptr[:, 64:128])

        # ---- filler machinery: labeled closures emitted into PE bubbles
        fillers = collections.deque()
        tile_ctr = {"n": 0}

        def pop_filler():
            tile_ctr["n"] += 1

        def flush_until(label):
            pass

        def flush_fillers():
            while fillers:
                _, fn = fillers.popleft()
                fn()

        # ---- projection for (b, half): 8 psum tiles contracted over 8 peers.
        # Emitted inline at points where the feeding collective is already
        # done (a proj matmul waiting on its rhs DMA would head-of-line
        # block the whole PE queue).
        def emit_proj(b, hh):
            rt_ = rhsp.tile([128, NCORES, NJ], BF16, name="rt", tag="rt")
            nc.sync.dma_start(out=rt_,
                              in_=a2a_out[b][hh].rearrange("c p t -> p c t"))
            rhs_tiles = [rt_[:, j, :] for j in range(NCORES)]
            for nt in range(NCC):
                py = filps.tile([128, NJ], F32, name="py", tag="q")
                for j in range(NCORES):
                    nc.tensor.matmul(py, wp[:, j, nt * 128:(nt + 1) * 128],
                                     rhs_tiles[j],
                                     start=(j == 0), stop=(j == NCORES - 1))
                yo = yop.tile([128, NJ], F32, name="yo", tag="yo")
                nc.vector.tensor_copy(yo, py)
                nc.sync.dma_start(
                    out=y_h[nt * 128:(nt + 1) * 128,
                            (b * 2 + hh) * NJ:(b * 2 + hh + 1) * NJ], in_=yo)

        # ---- one attention strip, software-pipelined with filler slots
        def emit_strip(b, st):
            i0 = st * NI
            njt = 2 * (st + 1)
            oaug = oaps.tile([65, 1024], F32, name="oaug", tag="oaug")
            weis = {}

            def emit_sco_exp(jt):
                j0 = jt * NJ
                d = jt - (njt - 2)
                lo = 128 if d == 1 else 0
                sco = scops.tile([128, 2 * NI], F32, name="sco", tag="sco")
                for h in range(2):
                    nc.tensor.matmul(
                        sco[:, h * NI + lo:(h + 1) * NI],
                        kt[b][:, h, j0:j0 + NJ],
                        qt[b][:, h, i0 + lo:i0 + NI],
                        start=True, stop=True,
                    )
                wei = weip.tile([128, 2 * NI], BF16, name="wei", tag="wei")
                if d < 1:
                    nc.scalar.activation(wei, sco,
                                         mybir.ActivationFunctionType.Exp,
                                         scale=SCALE)
                else:
                    for h in range(2):
                        nc.scalar.activation(
                            wei[:, h * NI + lo:(h + 1) * NI],
                            sco[:, h * NI + lo:(h + 1) * NI],
                            mybir.ActivationFunctionType.Exp, scale=SCALE)
                if d >= 0:
                    for h in range(2):
                        nc.vector.tensor_mul(
                            wei[:, h * NI + lo:(h + 1) * NI],
                            wei[:, h * NI + lo:(h + 1) * NI],
                            masks[:, d, lo:],
                        )
                weis[jt] = (wei, lo)

            def emit_av(jt):
                wei, lo = weis.pop(jt)
                for h in range(2):
                    nc.tensor.matmul(
                        oaug[:, h * 512 + lo:h * 512 + NI],
                        vaug[b][:, jt, h * 65:(h + 1) * 65],
                        wei[:, h * NI + lo:(h + 1) * NI],
                        start=(jt == 0), stop=(jt == njt - 1),
                    )

            for jt in range(njt):
                emit_sco_exp(jt)
                if jt >= 2:
                    pop_filler()
                    emit_av(jt - 2)
            emit_av(njt - 2)
            emit_av(njt - 1)

            # epilogue: normalize via reciprocal + PE-broadcast to 64 rows
            r = smallp.tile([1, 2 * NI], BF16, name="r", tag="r")
            with nc.allow_low_precision(reason="softmax denom recip to bf16"):
                for h in range(2):
                    nc.vector.reciprocal(r[:, h * NI:(h + 1) * NI],
                                         oaug[64:65, h * 512:h * 512 + NI])
            rb = filps.tile([64, 2 * NI], F32, name="rb", tag="q")
            nc.tensor.matmul(rb, ones64, r, start=True, stop=True)
            rbs = smallp.tile([64, 2 * NI], F32, name="rbs", tag="rbs")
            nc.vector.tensor_copy(rbs, rb)
            att = attp.tile([128, NI], BF16, name="att", tag="att")
            for h in range(2):
                nc.vector.tensor_mul(
                    att[h * 64:(h + 1) * 64, :],
                    oaug[0:64, h * 512:h * 512 + NI],
                    rbs[:, h * NI:(h + 1) * NI],
                )
            # ship the two 128-token chunks to their reshard slots
            hh = 0 if st < 4 else 1
            for hf in range(2):
                nc.sync.dma_start(
                    out=a2a_in[b][hh][(2 * st + hf) % 8],
                    in_=att[:, hf * NJ:(hf + 1) * NJ],
                )

        def emit_a2a(b, hh):
            nc.gpsimd.collective_compute(
                "AllToAll",
                mybir.AluOpType.bypass,
                replica_groups=[list(range(NCORES))],
                ins=[a2a_in[b][hh][:, :, :].opt()],
                outs=[a2a_out[b][hh][:, :, :].opt()],
            )

        # ================= main schedule =================
        # All QKV beyond (0,q0) rides the filler queue in dependency order:
        # (0,q1..q3) then (1,q0),(1,q1); b1's late quarters pop inside b1's
        # own early strips. flush_until() guarantees a quarter is emitted
        # before the first strip that reads it.
        def enq_qkv(b, tq):
            for which in ("q", "k", "v"):
                emit_qkv_unit(b, tq, which)

        for which in ("q", "k", "v"):
            emit_qkv_unit(0, 0, which)
        for st in range(NSTRIP):
            if st < 3:
                enq_qkv(0, st + 1)
            if 2 <= st <= 3:
                load_x_quarter(1, st - 2)
            if 3 <= st <= 4:
                enq_qkv(1, st - 3)
            if st >= 2:
                flush_until((0, st // 2))
            emit_strip(0, st)
            if st == 3:
                emit_a2a(0, 0)
        emit_a2a(0, 1)    # issue before the QKV(b1) flush: b0 data is ready
        flush_until((1, 0))
        flush_until((1, 1))

        for st in range(NSTRIP):
            if st == 0:
                load_x_quarter(1, 2)
                enq_qkv(1, 2)
            if st == 1:
                load_x_quarter(1, 3)
                enq_qkv(1, 3)
            if st >= 4:
                flush_until((1, st // 2))
            emit_strip(1, st)
            if st == 2:
                emit_proj(0, 0)   # cc(0,0) completed during b0's tail
            if st == 3:
                emit_a2a(1, 0)
            if st == 5:
                emit_proj(0, 1)   # cc(0,1) completed during b1's early strips
        # tail: proj(1,0) overlaps the last collective, then proj(1,1)
        emit_proj(1, 0)
        emit_a2a(1, 1)
        emit_proj(1, 1)
    return nc


_NC_CACHE = {}


def _get_nc():
    if "nc" not in _NC_CACHE:
        _NC_CACHE["nc"] = build_nc()
    return _NC_CACHE["nc"]


def _host_masks():
    jl = np.arange(NJ)[:, None]
    il = np.arange(NI)[None, :]
    return np.stack([(il >= jl + d * 128) for d in range(2)]).astype(ml_dtypes.bfloat16)


def kernel(x, Wk, Wq, Wv, Wp, bp):
    x = np.asarray(x)
    xT = np.ascontiguousarray(x.transpose(0, 2, 1)).astype(ml_dtypes.bfloat16)
    wpb = np.asarray(Wp).astype(ml_dtypes.bfloat16)
    masks = _host_masks()
    in_maps = []
    for c in range(NCORES):
        cs = slice(c * HPC, (c + 1) * HPC)
        in_maps.append({
            "xT": xT,
            "wq": np.ascontiguousarray(Wq[:, cs]).astype(ml_dtypes.bfloat16),
            "wk": np.ascontiguousarray(Wk[:, cs]).astype(ml_dtypes.bfloat16),
            "wv": np.ascontiguousarray(Wv[:, cs]).astype(ml_dtypes.bfloat16),
            "wp": wpb,
            "masks": masks,
        })
    res = run_bass_kernel_spmd(_get_nc(), in_maps, list(range(NCORES)))
    # core c's y_out[:, b, hh, :] covers batch-b tokens [1024*hh+128c, +128)
    yT = np.zeros((B, C, T), np.float32)
    for c in range(NCORES):
        yo = res.results[c]["y_out"].reshape(C, B, 2, NJ)
        for b in range(B):
            for hh in range(2):
                t0 = 1024 * hh + 128 * c
                yT[b, :, t0:t0 + 128] = yo[:, b, hh, :]
    y = yT.transpose(0, 2, 1) + np.asarray(bp)[None, None, :]
    return np.ascontiguousarray(y, dtype=np.float32)


# revision 39
# speedup vs baseline: 1.0630x; 1.0067x over previous
"""Multi-head causal attention (B=2,T=2048,C=1024,H=16,Dh=64) on 8 trn2 cores.

Sharding: tensor-parallel over heads - core c owns heads (2c, 2c+1).
Per core: QKV projections for its 128 q/k/v columns, causal flash attention
for its 2 heads x 2 batches, then an AllToAll reshard (heads-sharded ->
token-sharded) and the output projection for its own tokens.

v2 schedule (vs the 196us baseline):
- strips are 256 tokens; software-pipelined score->exp->AV with depth-1 skew
  so PE and Act overlap per tile.
- a "filler" queue feeds PE bubbles during Act-bound attention: QKV of the
  other batch and projection units run inside attention windows.
- each batch's reshard is split into two half collectives (token ownership
  interleaved at 128-token granularity); after the last attention strip only
  one small collective plus a quarter of the projection remains, and the
  second-to-last projection quarter runs inside that collective's shadow.
- collective APs keep their natural [8,128,128] chunk form.
- v transposes ride the DMA xbar instead of PE.
- QKV runs by token quarters so attention starts a few us into the kernel.
"""
import collections

import numpy as np
import ml_dtypes

import concourse.bass as bass
import concourse.mybir as mybir
import concourse.tile as tile
from concourse.bass_utils import run_bass_kernel_spmd
from concourse.masks import make_identity
from concourse.vector_clock import ScopedClock

BF16 = mybir.dt.bfloat16
F32 = mybir.dt.float32

B, T, C = 2, 2048, 1024
H, DH = 16, 64
NCORES = 8
HPC = 128      # head-columns per core (2 heads x 64)
NI = 256       # strip width (query tokens)
NJ = 128       # key-tile width
NSTRIP = T // NI          # 8 strips per batch
NJT = T // NJ             # 16 j-tiles per batch
NCC = C // 128            # 8 contraction chunks
NQ = 4                    # token quarters for QKV
QW = T // NQ              # 512 tokens per quarter
SCALE = DH ** -0.5


class TileContextP(tile.TileContext):
    """This walrus build caps sync waits at 1 per instruction (2 for
    EventSemaphore). Tile can emit more. Legalize by spilling excess waits
    onto same-engine nops emitted just before the instruction, and do the
    same for the kernel-tail drain."""

    def _commit_instruction(self, inst, lazy_reg_writes: bool = True):
        si = getattr(inst, "sync_info", None)
        if si is not None and si.on_wait:
            cap = 2 if isinstance(inst, mybir.InstEventSemaphore) else 1
            if len(si.on_wait) > cap:
                waits = list(si.on_wait)
                keep, spill = waits[:cap - 1] if cap > 1 else [], waits[cap - 1:]
                # keep the last wait on the inst, spill the rest
                spill, last = spill[:-1], spill[-1:]
                for w in spill:
                    nop = mybir.InstNoOp(
                        name=self.nc.get_next_instruction_name(),
                        engine=inst.engine,
                        sync_info=mybir.SyncInfo(on_wait=[w], on_update=[]),
                        bass_nofuse=True,
                    )
                    self._add_instruction(nop)
                si.on_wait = keep + last
        return super()._commit_instruction(inst, lazy_reg_writes)

    def _drain_and_barrier(self, tick_clock, wait_clock):
        probe = self.nc.sync.nop()
        wait_clock.add_sem_waits(
            probe.ins, ScopedClock({None: tick_clock.global_clock})
        )
        waits = list(probe.ins.sync_info.on_wait) if probe.ins.sync_info else []
        if probe.ins.sync_info:
            probe.ins.sync_info.on_wait = []
        for w in waits:
            n = self.nc.sync.nop()
            si = n.ins.sync_info
            if si is None:
                n.ins.sync_info = mybir.SyncInfo(on_wait=[w], on_update=[])
            else:
                si.on_wait = [w]
        self.nc.sync.drain()
        self.nc.all_engine_barrier()
        assert self.sems is not None
        popped = self.nc._tile_sem_poison_stack.pop()
        assert popped is self._sem_poison
        self.nc.clear_and_free_semaphores(list(self.sems.allocated().values()))
        self.nc.all_engine_barrier()


def build_nc():
    nc = bass.Bass()
    xT_h = nc.dram_tensor("xT", [B, C, T], BF16, kind="ExternalInput")
    wq_h = nc.dram_tensor("wq", [C, HPC], BF16, kind="ExternalInput")
    wk_h = nc.dram_tensor("wk", [C, HPC], BF16, kind="ExternalInput")
    wv_h = nc.dram_tensor("wv", [C, HPC], BF16, kind="ExternalInput")
    wp_h = nc.dram_tensor("wp", [C, C], BF16, kind="ExternalInput")
    mk_h = nc.dram_tensor("masks", [2, NJ, NI], BF16, kind="ExternalInput")
    # y_out[:, b, hh, :] = out rows x tokens [1024*hh + 128*core, +128) of b
    y_h = nc.dram_tensor("y_out", [C, B * 2 * NJ], F32, kind="ExternalOutput")
    # reshard buffers, one per (batch, half): slot j of a2a_in = this core's
    # head-pair rows for tokens [1024*hh + 128*j, +128)
    a2a_in = [[nc.dram_tensor(f"a2a_in{b}_{hh}", [NCORES, HPC, NJ], BF16)
               for hh in range(2)] for b in range(B)]
    a2a_out = [[nc.dram_tensor(f"a2a_out{b}_{hh}", [NCORES, HPC, NJ], BF16)
                for hh in range(2)] for b in range(B)]

    with TileContextP(nc) as tc, \
         tc.tile_pool(name="singles", bufs=1) as singles, \
         tc.tile_pool(name="xqp", bufs=6) as xqp, \
         tc.tile_pool(name="qkp", bufs=1) as qkp, \
         tc.tile_pool(name="vtp", bufs=2) as vtp, \
         tc.tile_pool(name="weip", bufs=5) as weip, \
         tc.tile_pool(name="attp", bufs=3) as attp, \
         tc.tile_pool(name="smallp", bufs=2) as smallp, \
         tc.tile_pool(name="rhsp", bufs=2) as rhsp, \
         tc.tile_pool(name="yop", bufs=4) as yop, \
         tc.tile_pool(name="scops", bufs=2, space="PSUM") as scops, \
         tc.tile_pool(name="oaps", bufs=2, space="PSUM") as oaps, \
         tc.tile_pool(name="filps", bufs=2, space="PSUM") as filps:

        # ---- weights first (gate the first matmuls), then x(b0), masks
        wq = singles.tile([128, NCC, HPC], BF16, name="wq_s", tag="wq_s")
        wk = singles.tile([128, NCC, HPC], BF16, name="wk_s", tag="wk_s")
        wv = singles.tile([128, NCC, HPC], BF16, name="wv_s", tag="wv_s")
        nc.scalar.dma_start(out=wq, in_=wq_h.rearrange("(n p) m -> p n m", p=128))

        xsrc = [xT_h[b].rearrange("(n p) t -> p n t", p=128) for b in range(B)]
        xt = {}

        def load_x_quarter(b, tq):
            xq = xqp.tile([128, NCC, QW], BF16, name="xq", tag="xq")
            for cc in range(NCC):
                nc.sync.dma_start(
                    out=xq[:, cc, :],
                    in_=xsrc[b][:, cc, tq * QW:(tq + 1) * QW],
                )
            xt[(b, tq)] = xq

        load_x_quarter(0, 0)
        nc.scalar.dma_start(out=wk, in_=wk_h.rearrange("(n p) m -> p n m", p=128))
        nc.scalar.dma_start(out=wv, in_=wv_h.rearrange("(n p) m -> p n m", p=128))
        masks = singles.tile([128, 2, NI], BF16, name="masks_s", tag="masks_s")
        nc.scalar.dma_start(out=masks, in_=mk_h.rearrange("d p i -> p d i"))
        for tq in range(1, NQ):
            load_x_quarter(0, tq)
        wp = singles.tile([128, NCC, C], BF16, name="wp_s", tag="wp_s")
        nc.sync.dma_start(out=wp, in_=wp_h.rearrange("(n p) m -> p n m", p=128))

        ident = singles.tile([128, 128], BF16, name="ident_s", tag="ident_s")
        make_identity(nc, ident)
        ones64 = singles.tile([1, 64], BF16, name="ones64", tag="ones64")
        nc.vector.memset(ones64, 1.0)

        qt = {b: qkp.tile([64, 2, T], BF16, name=f"qt{b}", tag=f"qt{b}")
              for b in range(B)}
        kt = {b: qkp.tile([64, 2, T], BF16, name=f"kt{b}", tag=f"kt{b}")
              for b in range(B)}
        vaug = {b: qkp.tile([128, NJT, 130], BF16, name=f"vaug{b}", tag=f"vaug{b}")
                for b in range(B)}
        for b in range(B):
            nc.vector.memset(vaug[b], 1.0)

        def emit_qkv_unit(b, tq, which):
            """One (w, quarter) projection: 8 accum matmuls + copy-out.
            For v, also xbar-transpose into vaug."""
            xq = xt[(b, tq)]
            w_t = {"q": wq, "k": wk, "v": wv}[which]
            ps = filps.tile([128, QW], F32, name="psq", tag="q")
            for cc in range(NCC):
                nc.tensor.matmul(ps, w_t[:, cc, :], xq[:, cc, :],
                                 start=(cc == 0), stop=(cc == NCC - 1))
            if which == "q":
                for h in range(2):
                    nc.vector.tensor_copy(
                        qt[b][:, h, tq * QW:(tq + 1) * QW],
                        ps[h * 64:(h + 1) * 64, :])
            elif which == "k":
                for h in range(2):
                    nc.scalar.copy(
                        kt[b][:, h, tq * QW:(tq + 1) * QW],
                        ps[h * 64:(h + 1) * 64, :])
            else:
                vt = vtp.tile([128, QW], BF16, name="vt", tag="vt")
                nc.vector.tensor_copy(vt, ps)
                for j in range(QW // NJ):
                    jt = tq * (QW // NJ) + j
                    ptr = filps.tile([128, NJ], BF16, name="ptr", tag="q")
                    nc.tensor.transpose(ptr, vt[:, j * NJ:(j + 1) * NJ], ident)
                    nc.vector.tensor_copy(vaug[b][:, jt, 0:64], ptr[:, 0:64])
                    nc.vector.tensor_copy(vaug[b][:, jt, 65:129], ptr[:, 64:128])

        # ---- filler machinery: labeled closures emitted into PE bubbles
        fillers = collections.deque()
        tile_ctr = {"n": 0}

        def pop_filler():
            tile_ctr["n"] += 1

        def flush_until(label):
            pass

        def flush_fillers():
            while fillers:
                _, fn = fillers.popleft()
                fn()

        # ---- projection for (b, half): 8 psum tiles contracted over 8 peers.
        # Emitted inline at points where the feeding collective is already
        # done (a proj matmul waiting on its rhs DMA would head-of-line
        # block the whole PE queue).
        def emit_proj(b, hh):
            rt_ = rhsp.tile([128, NCORES, NJ], BF16, name="rt", tag="rt")
            nc.sync.dma_start(out=rt_,
                              in_=a2a_out[b][hh].rearrange("c p t -> p c t"))
            rhs_tiles = [rt_[:, j, :] for j in range(NCORES)]
            for nt in range(NCC):
                py = filps.tile([128, NJ], F32, name="py", tag="q")
                for j in range(NCORES):
                    nc.tensor.matmul(py, wp[:, j, nt * 128:(nt + 1) * 128],
                                     rhs_tiles[j],
                                     start=(j == 0), stop=(j == NCORES - 1))
                yo = yop.tile([128, NJ], F32, name="yo", tag="yo")
                nc.vector.tensor_copy(yo, py)
                nc.sync.dma_start(
                    out=y_h[nt * 128:(nt + 1) * 128,
                            (b * 2 + hh) * NJ:(b * 2 + hh + 1) * NJ], in_=yo)

        # ---- one attention strip, software-pipelined with filler slots
        def emit_strip(b, st):
            i0 = st * NI
            njt = 2 * (st + 1)
            oaug = oaps.tile([65, 1024], F32, name="oaug", tag="oaug")
            weis = {}

            def emit_sco_exp(jt):
                j0 = jt * NJ
                d = jt - (njt - 2)
                lo = 128 if d == 1 else 0
                sco = scops.tile([128, 2 * NI], F32, name="sco", tag="sco")
                for h in range(2):
                    nc.tensor.matmul(
                        sco[:, h * NI + lo:(h + 1) * NI],
                        kt[b][:, h, j0:j0 + NJ],
                        qt[b][:, h, i0 + lo:i0 + NI],
                        start=True, stop=True,
                    )
                wei = weip.tile([128, 2 * NI], BF16, name="wei", tag="wei")
                if d < 1:
                    nc.scalar.activation(wei, sco,
                                         mybir.ActivationFunctionType.Exp,
                                         scale=SCALE)
                else:
                    for h in range(2):
                        nc.scalar.activation(
                            wei[:, h * NI + lo:(h + 1) * NI],
                            sco[:, h * NI + lo:(h + 1) * NI],
                            mybir.ActivationFunctionType.Exp, scale=SCALE)
                if d >= 0:
                    for h in range(2):
                        nc.vector.tensor_mul(
                            wei[:, h * NI + lo:(h + 1) * NI],
                            wei[:, h * NI + lo:(h + 1) * NI],
                            masks[:, d, lo:],
                        )
                weis[jt] = (wei, lo)

            def emit_av(jt):
                wei, lo = weis.pop(jt)
                for h in range(2):
                    nc.tensor.matmul(
                        oaug[:, h * 512 + lo:h * 512 + NI],
                        vaug[b][:, jt, h * 65:(h + 1) * 65],
                        wei[:, h * NI + lo:(h + 1) * NI],
                        start=(jt == 0), stop=(jt == njt - 1),
                    )

            for jt in range(njt):
                emit_sco_exp(jt)
                if jt >= 2:
                    pop_filler()
                    emit_av(jt - 2)
            emit_av(njt - 2)
            emit_av(njt - 1)

            # epilogue: normalize via reciprocal + PE-broadcast to 64 rows
            r = smallp.tile([1, 2 * NI], BF16, name="r", tag="r")
            with nc.allow_low_precision(reason="softmax denom recip to bf16"):
                for h in range(2):
                    nc.vector.reciprocal(r[:, h * NI:(h + 1) * NI],
                                         oaug[64:65, h * 512:h * 512 + NI])
            rb = filps.tile([64, 2 * NI], F32, name="rb", tag="q")
            nc.tensor.matmul(rb, ones64, r, start=True, stop=True)
            rbs = smallp.tile([64, 2 * NI], F32, name="rbs", tag="rbs")
            nc.vector.tensor_copy(rbs, rb)
            att = attp.tile([128, NI], BF16, name="att", tag="att")
            for h in range(2):
                nc.vector.tensor_mul(
                    att[h * 64:(h + 1) * 64, :],
                    oaug[0:64, h * 512:h * 512 + NI],
                    rbs[:, h * NI:(h + 1) * NI],
                )
            # ship the two 128-token chunks to their reshard slots
            hh = 0 if st < 4 else 1
            for hf in range(2):
                nc.sync.dma_start(
                    out=a2a_in[b][hh][(2 * st + hf) % 8],
                    in_=att[:, hf * NJ:(hf + 1) * NJ],
                )

        def emit_a2a(b, hh):
            nc.gpsimd.collective_compute(
                "AllToAll",
                mybir.AluOpType.bypass,
                replica_groups=[list(range(NCORES))],
                ins=[a2a_in[b][hh][:, :, :]],
                outs=[a2a_out[b][hh][:, :, :]],
            )

        # ================= main schedule =================
        # All QKV beyond (0,q0) rides the filler queue in dependency order:
        # (0,q1..q3) then (1,q0),(1,q1); b1's late quarters pop inside b1's
        # own early strips. flush_until() guarantees a quarter is emitted
        # before the first strip that reads it.
        def enq_qkv(b, tq):
            for which in ("q", "k", "v"):
                emit_qkv_unit(b, tq, which)

        for which in ("q", "k", "v"):
            emit_qkv_unit(0, 0, which)
        for st in range(NSTRIP):
            if st < 3:
                enq_qkv(0, st + 1)
            if 2 <= st <= 3:
                load_x_quarter(1, st - 2)
            if 3 <= st <= 4:
                enq_qkv(1, st - 3)
            if st >= 2:
                flush_until((0, st // 2))
            emit_strip(0, st)
            if st == 3:
                emit_a2a(0, 0)
        emit_a2a(0, 1)    # issue before the QKV(b1) flush: b0 data is ready
        flush_until((1, 0))
        flush_until((1, 1))

        for st in range(NSTRIP):
            if st == 0:
                load_x_quarter(1, 2)
                enq_qkv(1, 2)
            if st == 1:
                load_x_quarter(1, 3)
                enq_qkv(1, 3)
            if st >= 4:
                flush_until((1, st // 2))
            emit_strip(1, st)
            if st == 2:
                emit_proj(0, 0)   # cc(0,0) completed during b0's tail
            if st == 3:
                emit_a2a(1, 0)
            if st == 5:
                emit_proj(0, 1)   # cc(0,1) completed during b1's early strips
        # tail: proj(1,0) overlaps the last collective, then proj(1,1)
        emit_proj(1, 0)
        emit_a2a(1, 1)
        emit_proj(1, 1)
    return nc


_NC_CACHE = {}


def _get_nc():
    if "nc" not in _NC_CACHE:
        _NC_CACHE["nc"] = build_nc()
    return _NC_CACHE["nc"]


def _host_masks():
    jl = np.arange(NJ)[:, None]
    il = np.arange(NI)[None, :]
    return np.stack([(il >= jl + d * 128) for d in range(2)]).astype(ml_dtypes.bfloat16)


def kernel(x, Wk, Wq, Wv, Wp, bp):
    x = np.asarray(x)
    xT = np.ascontiguousarray(x.transpose(0, 2, 1)).astype(ml_dtypes.bfloat16)
    wpb = np.asarray(Wp).astype(ml_dtypes.bfloat16)
    masks = _host_masks()
    in_maps = []
    for c in range(NCORES):
        cs = slice(c * HPC, (c + 1) * HPC)
        in_maps.append({
            "xT": xT,
            "wq": np.ascontiguousarray(Wq[:, cs]).astype(ml_dtypes.bfloat16),
            "wk": np.ascontiguousarray(Wk[:, cs]).astype(ml_dtypes.bfloat16),
            "wv": np.ascontiguousarray(Wv[:, cs]).astype(ml_dtypes.bfloat16),
            "wp": wpb,
            "masks": masks,
        })
    res = run_bass_kernel_spmd(_get_nc(), in_maps, list(range(NCORES)))
    # core c's y_out[:, b, hh, :] covers batch-b tokens [1024*hh+128c, +128)
    yT = np.zeros((B, C, T), np.float32)
    for c in range(NCORES):
        yo = res.results[c]["y_out"].reshape(C, B, 2, NJ)
        for b in range(B):
            for hh in range(2):
                t0 = 1024 * hh + 128 * c
                yT[b, :, t0:t0 + 128] = yo[:, b, hh, :]
    y = yT.transpose(0, 2, 1) + np.asarray(bp)[None, None, :]
    return np.ascontiguousarray(y, dtype=np.float32)


# revision 40
# speedup vs baseline: 1.1325x; 1.0654x over previous
"""Multi-head causal attention (B=2,T=2048,C=1024,H=16,Dh=64) on 8 trn2 cores.

Sharding: tensor-parallel over heads - core c owns heads (2c, 2c+1).
Per core: QKV projections for its 128 q/k/v columns, causal flash attention
for its 2 heads x 2 batches, then an AllToAll reshard (heads-sharded ->
token-sharded) and the output projection for its own tokens.

v2 schedule (vs the 196us baseline):
- strips are 256 tokens; software-pipelined score->exp->AV with depth-1 skew
  so PE and Act overlap per tile.
- a "filler" queue feeds PE bubbles during Act-bound attention: QKV of the
  other batch and projection units run inside attention windows.
- each batch's reshard is split into two half collectives (token ownership
  interleaved at 128-token granularity); after the last attention strip only
  one small collective plus a quarter of the projection remains, and the
  second-to-last projection quarter runs inside that collective's shadow.
- collective APs keep their natural [8,128,128] chunk form.
- v transposes ride the DMA xbar instead of PE.
- QKV runs by token quarters so attention starts a few us into the kernel.
"""
import collections

import numpy as np
import ml_dtypes

import concourse.bass as bass
import concourse.mybir as mybir
import concourse.tile as tile
from concourse.bass_utils import run_bass_kernel_spmd
from concourse.masks import make_identity
from concourse.vector_clock import ScopedClock

BF16 = mybir.dt.bfloat16
F32 = mybir.dt.float32

B, T, C = 2, 2048, 1024
H, DH = 16, 64
NCORES = 8
HPC = 128      # head-columns per core (2 heads x 64)
NI = 256       # strip width (query tokens)
NJ = 128       # key-tile width
NSTRIP = T // NI          # 8 strips per batch
NJT = T // NJ             # 16 j-tiles per batch
NCC = C // 128            # 8 contraction chunks
NQ = 4                    # token quarters for QKV
QW = T // NQ              # 512 tokens per quarter
SCALE = DH ** -0.5


class TileContextP(tile.TileContext):
    """This walrus build caps sync waits at 1 per instruction (2 for
    EventSemaphore). Tile can emit more. Legalize by spilling excess waits
    onto same-engine nops emitted just before the instruction, and do the
    same for the kernel-tail drain."""

    def _commit_instruction(self, inst, lazy_reg_writes: bool = True):
        si = getattr(inst, "sync_info", None)
        if si is not None and si.on_wait:
            cap = 2 if isinstance(inst, mybir.InstEventSemaphore) else 1
            if len(si.on_wait) > cap:
                waits = list(si.on_wait)
                keep, spill = waits[:cap - 1] if cap > 1 else [], waits[cap - 1:]
                # keep the last wait on the inst, spill the rest
                spill, last = spill[:-1], spill[-1:]
                for w in spill:
                    nop = mybir.InstNoOp(
                        name=self.nc.get_next_instruction_name(),
                        engine=inst.engine,
                        sync_info=mybir.SyncInfo(on_wait=[w], on_update=[]),
                        bass_nofuse=True,
                    )
                    self._add_instruction(nop)
                si.on_wait = keep + last
        return super()._commit_instruction(inst, lazy_reg_writes)

    def _drain_and_barrier(self, tick_clock, wait_clock):
        probe = self.nc.sync.nop()
        wait_clock.add_sem_waits(
            probe.ins, ScopedClock({None: tick_clock.global_clock})
        )
        waits = list(probe.ins.sync_info.on_wait) if probe.ins.sync_info else []
        if probe.ins.sync_info:
            probe.ins.sync_info.on_wait = []
        for w in waits:
            n = self.nc.sync.nop()
            si = n.ins.sync_info
            if si is None:
                n.ins.sync_info = mybir.SyncInfo(on_wait=[w], on_update=[])
            else:
                si.on_wait = [w]
        self.nc.sync.drain()
        self.nc.all_engine_barrier()
        assert self.sems is not None
        popped = self.nc._tile_sem_poison_stack.pop()
        assert popped is self._sem_poison
        self.nc.clear_and_free_semaphores(list(self.sems.allocated().values()))
        self.nc.all_engine_barrier()


def build_nc():
    nc = bass.Bass()
    xT_h = nc.dram_tensor("xT", [B, C, T], BF16, kind="ExternalInput")
    wq_h = nc.dram_tensor("wq", [C, HPC], BF16, kind="ExternalInput")
    wk_h = nc.dram_tensor("wk", [C, HPC], BF16, kind="ExternalInput")
    wv_h = nc.dram_tensor("wv", [C, HPC], BF16, kind="ExternalInput")
    wp_h = nc.dram_tensor("wp", [C, C], BF16, kind="ExternalInput")
    mk_h = nc.dram_tensor("masks", [2, NJ, NI], BF16, kind="ExternalInput")
    # y_out[:, b, hh, :] = out rows x tokens [1024*hh + 128*core, +128) of b
    y_h = nc.dram_tensor("y_out", [C, B * 2 * NJ], F32, kind="ExternalOutput")
    # reshard buffers, one per (batch, half): slot j of a2a_in = this core's
    # head-pair rows for tokens [1024*hh + 128*j, +128)
    a2a_in = [[nc.dram_tensor(f"a2a_in{b}_{hh}", [NCORES, HPC, NJ], BF16)
               for hh in range(2)] for b in range(B)]
    a2a_out = [[nc.dram_tensor(f"a2a_out{b}_{hh}", [NCORES, HPC, NJ], BF16)
                for hh in range(2)] for b in range(B)]

    with TileContextP(nc) as tc, \
         tc.tile_pool(name="singles", bufs=1) as singles, \
         tc.tile_pool(name="xqp", bufs=6) as xqp, \
         tc.tile_pool(name="qkp", bufs=1) as qkp, \
         tc.tile_pool(name="vtp", bufs=2) as vtp, \
         tc.tile_pool(name="weip", bufs=5) as weip, \
         tc.tile_pool(name="attp", bufs=3) as attp, \
         tc.tile_pool(name="smallp", bufs=2) as smallp, \
         tc.tile_pool(name="rhsp", bufs=2) as rhsp, \
         tc.tile_pool(name="yop", bufs=4) as yop, \
         tc.tile_pool(name="scops", bufs=2, space="PSUM") as scops, \
         tc.tile_pool(name="oaps", bufs=2, space="PSUM") as oaps, \
         tc.tile_pool(name="filps", bufs=2, space="PSUM") as filps:

        # ---- weights first (gate the first matmuls), then x(b0), masks
        wq = singles.tile([128, NCC, HPC], BF16, name="wq_s", tag="wq_s")
        wk = singles.tile([128, NCC, HPC], BF16, name="wk_s", tag="wk_s")
        wv = singles.tile([128, NCC, HPC], BF16, name="wv_s", tag="wv_s")
        nc.scalar.dma_start(out=wq, in_=wq_h.rearrange("(n p) m -> p n m", p=128))

        xsrc = [xT_h[b].rearrange("(n p) t -> p n t", p=128) for b in range(B)]
        xt = {}

        def load_x_quarter(b, tq):
            xq = xqp.tile([128, NCC, QW], BF16, name="xq", tag="xq")
            for cc in range(NCC):
                nc.sync.dma_start(
                    out=xq[:, cc, :],
                    in_=xsrc[b][:, cc, tq * QW:(tq + 1) * QW],
                )
            xt[(b, tq)] = xq

        load_x_quarter(0, 0)
        nc.scalar.dma_start(out=wk, in_=wk_h.rearrange("(n p) m -> p n m", p=128))
        nc.scalar.dma_start(out=wv, in_=wv_h.rearrange("(n p) m -> p n m", p=128))
        masks = singles.tile([128, 2, NI], BF16, name="masks_s", tag="masks_s")
        nc.scalar.dma_start(out=masks, in_=mk_h.rearrange("d p i -> p d i"))
        for tq in range(1, NQ):
            load_x_quarter(0, tq)
        wp = singles.tile([128, NCC, C], BF16, name="wp_s", tag="wp_s")
        nc.sync.dma_start(out=wp, in_=wp_h.rearrange("(n p) m -> p n m", p=128))

        ident = singles.tile([128, 128], BF16, name="ident_s", tag="ident_s")
        make_identity(nc, ident)
        ones64 = singles.tile([1, 64], BF16, name="ones64", tag="ones64")
        nc.vector.memset(ones64, 1.0)

        qt = {b: qkp.tile([64, 2, T], BF16, name=f"qt{b}", tag=f"qt{b}")
              for b in range(B)}
        kt = {b: qkp.tile([64, 2, T], BF16, name=f"kt{b}", tag=f"kt{b}")
              for b in range(B)}
        vaug = {b: qkp.tile([128, NJT, 130], BF16, name=f"vaug{b}", tag=f"vaug{b}")
                for b in range(B)}
        for b in range(B):
            nc.vector.memset(vaug[b], 1.0)

        def emit_qkv_unit(b, tq, which):
            """One (w, quarter) projection: 8 accum matmuls + copy-out.
            For v, also xbar-transpose into vaug."""
            xq = xt[(b, tq)]
            w_t = {"q": wq, "k": wk, "v": wv}[which]
            ps = filps.tile([128, QW], F32, name="psq", tag="q")
            for cc in range(NCC):
                nc.tensor.matmul(ps, w_t[:, cc, :], xq[:, cc, :],
                                 start=(cc == 0), stop=(cc == NCC - 1))
            if which == "q":
                for h in range(2):
                    nc.vector.tensor_copy(
                        qt[b][:, h, tq * QW:(tq + 1) * QW],
                        ps[h * 64:(h + 1) * 64, :])
            elif which == "k":
                for h in range(2):
                    nc.scalar.copy(
                        kt[b][:, h, tq * QW:(tq + 1) * QW],
                        ps[h * 64:(h + 1) * 64, :])
            else:
                vt = vtp.tile([128, QW], BF16, name="vt", tag="vt")
                nc.vector.tensor_copy(vt, ps)
                for j in range(QW // NJ):
                    jt = tq * (QW // NJ) + j
                    ptr = filps.tile([128, NJ], BF16, name="ptr", tag="q")
                    nc.tensor.transpose(ptr, vt[:, j * NJ:(j + 1) * NJ], ident)
                    nc.vector.tensor_copy(vaug[b][:, jt, 0:64], ptr[:, 0:64])
                    nc.vector.tensor_copy(vaug[b][:, jt, 65:129], ptr[:, 64:128])

        # ---- filler machinery: labeled closures emitted into PE bubbles
        fillers = collections.deque()
        tile_ctr = {"n": 0}

        def pop_filler():
            tile_ctr["n"] += 1

        def flush_until(label):
            pass

        def flush_fillers():
            while fillers:
                _, fn = fillers.popleft()
                fn()

        # ---- projection for (b, half): 8 psum tiles contracted over 8 peers.
        # Emitted inline at points where the feeding collective is already
        # done (a proj matmul waiting on its rhs DMA would head-of-line
        # block the whole PE queue).
        def emit_proj(b, hh, as_fillers=False):
            rt_ = rhsp.tile([128, NCORES, NJ], BF16, name="rt", tag="rt")
            nc.sync.dma_start(out=rt_,
                              in_=a2a_out[b][hh].rearrange("c p t -> p c t"))
            rhs_tiles = [rt_[:, j, :] for j in range(NCORES)]

            def unit(nt):
                py = filps.tile([128, NJ], F32, name="py", tag="q")
                for j in range(NCORES):
                    nc.tensor.matmul(py, wp[:, j, nt * 128:(nt + 1) * 128],
                                     rhs_tiles[j],
                                     start=(j == 0), stop=(j == NCORES - 1))
                yo = yop.tile([128, NJ], F32, name="yo", tag="yo")
                nc.vector.tensor_copy(yo, py)
                nc.sync.dma_start(
                    out=y_h[nt * 128:(nt + 1) * 128,
                            (b * 2 + hh) * NJ:(b * 2 + hh + 1) * NJ], in_=yo)
            for nt in range(NCC):
                if as_fillers:
                    fillers.append((("proj", b, hh),
                                    lambda nt_=nt: unit(nt_)))
                else:
                    unit(nt)

        # ---- one attention strip, software-pipelined with filler slots
        def emit_strip(b, st):
            i0 = st * NI
            njt = 2 * (st + 1)
            oaug = oaps.tile([65, 1024], F32, name="oaug", tag="oaug")
            weis = {}

            def emit_sco_exp(jt):
                j0 = jt * NJ
                d = jt - (njt - 2)
                lo = 128 if d == 1 else 0
                sco = scops.tile([128, 2 * NI], F32, name="sco", tag="sco")
                for h in range(2):
                    nc.tensor.matmul(
                        sco[:, h * NI + lo:(h + 1) * NI],
                        kt[b][:, h, j0:j0 + NJ],
                        qt[b][:, h, i0 + lo:i0 + NI],
                        start=True, stop=True,
                    )
                wei = weip.tile([128, 2 * NI], BF16, name="wei", tag="wei")
                if d < 1:
                    nc.scalar.activation(wei, sco,
                                         mybir.ActivationFunctionType.Exp,
                                         scale=SCALE)
                else:
                    for h in range(2):
                        nc.scalar.activation(
                            wei[:, h * NI + lo:(h + 1) * NI],
                            sco[:, h * NI + lo:(h + 1) * NI],
                            mybir.ActivationFunctionType.Exp, scale=SCALE)
                if d >= 0:
                    for h in range(2):
                        nc.vector.tensor_mul(
                            wei[:, h * NI + lo:(h + 1) * NI],
                            wei[:, h * NI + lo:(h + 1) * NI],
                            masks[:, d, lo:],
                        )
                weis[jt] = (wei, lo)

            def emit_av(jt):
                wei, lo = weis.pop(jt)
                for h in range(2):
                    nc.tensor.matmul(
                        oaug[:, h * 512 + lo:h * 512 + NI],
                        vaug[b][:, jt, h * 65:(h + 1) * 65],
                        wei[:, h * NI + lo:(h + 1) * NI],
                        start=(jt == 0), stop=(jt == njt - 1),
                    )

            for jt in range(njt):
                emit_sco_exp(jt)
                if jt >= 2:
                    pop_filler()
                    emit_av(jt - 2)
            emit_av(njt - 2)
            emit_av(njt - 1)

            # epilogue: normalize via reciprocal + PE-broadcast to 64 rows
            r = smallp.tile([1, 2 * NI], BF16, name="r", tag="r")
            with nc.allow_low_precision(reason="softmax denom recip to bf16"):
                for h in range(2):
                    nc.vector.reciprocal(r[:, h * NI:(h + 1) * NI],
                                         oaug[64:65, h * 512:h * 512 + NI])
            rb = filps.tile([64, 2 * NI], F32, name="rb", tag="q")
            nc.tensor.matmul(rb, ones64, r, start=True, stop=True)
            rbs = smallp.tile([64, 2 * NI], F32, name="rbs", tag="rbs")
            nc.vector.tensor_copy(rbs, rb)
            att = attp.tile([128, NI], BF16, name="att", tag="att")
            for h in range(2):
                nc.vector.tensor_mul(
                    att[h * 64:(h + 1) * 64, :],
                    oaug[0:64, h * 512:h * 512 + NI],
                    rbs[:, h * NI:(h + 1) * NI],
                )
            # ship the two 128-token chunks to their reshard slots
            hh = 0 if st < 4 else 1
            for hf in range(2):
                nc.sync.dma_start(
                    out=a2a_in[b][hh][(2 * st + hf) % 8],
                    in_=att[:, hf * NJ:(hf + 1) * NJ],
                )

        def emit_a2a(b, hh):
            nc.gpsimd.collective_compute(
                "AllToAll",
                mybir.AluOpType.bypass,
                replica_groups=[list(range(NCORES))],
                ins=[a2a_in[b][hh][:, :, :]],
                outs=[a2a_out[b][hh][:, :, :]],
            )

        # ================= main schedule =================
        # All QKV beyond (0,q0) rides the filler queue in dependency order:
        # (0,q1..q3) then (1,q0),(1,q1); b1's late quarters pop inside b1's
        # own early strips. flush_until() guarantees a quarter is emitted
        # before the first strip that reads it.
        def enq_qkv(b, tq):
            for which in ("q", "k", "v"):
                emit_qkv_unit(b, tq, which)

        for which in ("q", "k", "v"):
            emit_qkv_unit(0, 0, which)
        for st in range(NSTRIP):
            if st < 3:
                enq_qkv(0, st + 1)
            if 2 <= st <= 3:
                load_x_quarter(1, st - 2)
            if 3 <= st <= 4:
                enq_qkv(1, st - 3)
            if st >= 2:
                flush_until((0, st // 2))
            emit_strip(0, st)
            if st == 3:
                emit_a2a(0, 0)
        emit_a2a(0, 1)    # issue before the QKV(b1) flush: b0 data is ready
        flush_until((1, 0))
        flush_until((1, 1))

        for st in range(NSTRIP):
            if st == 0:
                load_x_quarter(1, 2)
                enq_qkv(1, 2)
            if st == 1:
                load_x_quarter(1, 3)
                enq_qkv(1, 3)
            if st >= 4:
                flush_until((1, st // 2))
            if st == 2:
                emit_proj(0, 0, as_fillers=True)
            if st == 6:
                emit_proj(0, 1, as_fillers=True)
            emit_strip(1, st)
            if st == 3:
                emit_a2a(1, 0)
        # tail: proj(1,0) overlaps the last collective, then proj(1,1)
        flush_fillers()
        emit_proj(1, 0)
        emit_a2a(1, 1)
        emit_proj(1, 1)
    return nc


_NC_CACHE = {}


def _get_nc():
    if "nc" not in _NC_CACHE:
        _NC_CACHE["nc"] = build_nc()
    return _NC_CACHE["nc"]


def _host_masks():
    jl = np.arange(NJ)[:, None]
    il = np.arange(NI)[None, :]
    return np.stack([(il >= jl + d * 128) for d in range(2)]).astype(ml_dtypes.bfloat16)


def kernel(x, Wk, Wq, Wv, Wp, bp):
    x = np.asarray(x)
    xT = np.ascontiguousarray(x.transpose(0, 2, 1)).astype(ml_dtypes.bfloat16)
    wpb = np.asarray(Wp).astype(ml_dtypes.bfloat16)
    masks = _host_masks()
    in_maps = []
    for c in range(NCORES):
        cs = slice(c * HPC, (c + 1) * HPC)
        in_maps.append({
            "xT": xT,
            "wq": np.ascontiguousarray(Wq[:, cs]).astype(ml_dtypes.bfloat16),
            "wk": np.ascontiguousarray(Wk[:, cs]).astype(ml_dtypes.bfloat16),
            "wv": np.ascontiguousarray(Wv[:, cs]).astype(ml_dtypes.bfloat16),
            "wp": wpb,
            "masks": masks,
        })
    res = run_bass_kernel_spmd(_get_nc(), in_maps, list(range(NCORES)))
    # core c's y_out[:, b, hh, :] covers batch-b tokens [1024*hh+128c, +128)
    yT = np.zeros((B, C, T), np.float32)
    for c in range(NCORES):
        yo = res.results[c]["y_out"].reshape(C, B, 2, NJ)
        for b in range(B):
            for hh in range(2):
                t0 = 1024 * hh + 128 * c
                yT[b, :, t0:t0 + 128] = yo[:, b, hh, :]
    y = yT.transpose(0, 2, 1) + np.asarray(bp)[None, None, :]
    return np.ascontiguousarray(y, dtype=np.float32)
